# revision 26
# baseline (speedup 1.0000x reference)
"""Trainium2 Bass kernel for nn_MemoryAttention (dense_transformer).

Strategy: shard query-time T across the 8 cores (T_loc = 256). Everything
else (weights, key, value) is replicated, so there are no collectives and
all outputs are disjoint slices. On each core, all activations are kept in
a transposed feature-on-partitions layout so every matmul feeds the next
one without any transposes:

  kT   = Wk^T-proj of key   : (E on partitions, S free)    [replicated work]
  qT   = Wq^T-proj of query : (E on partitions, T_loc free)
  LT_h = logits^T per head  : (S on partitions, T_loc free) = kT_h^T-contracted
  probsT = exp(LT * 1/8)    : softmax numerator
  attn^T = v_ext^T @ probsT : v_ext has a ones column -> row 64 = denominator
  out^T  = Wo^T @ attn^T    : (E on partitions, (b,t) free)

The softmax normalization (division by the denominator) is applied with a
rank-1 "broadcast" matmul (ones x recip) since partitions cannot be
broadcast on the vector engine. Denominators of 4 heads are gathered into
one PSUM tile with K=1 matmuls so one reciprocal op serves 4 heads.
"""

import os
import numpy as np

# Problem constants (hardcoded per contract)
T, S, B, QD, KD, VD, E, H = 2048, 2048, 2, 1024, 1024, 64, 1024, 16
DH = E // H            # 64
NCORES = 8
TLOC = T // NCORES     # 256
SCALE = DH ** -0.5     # 0.125

_CACHE = {}


def _build_program():
    import concourse.tile as tile
    from concourse import bacc, mybir

    f32 = mybir.dt.float32
    f32r = mybir.dt.float32r
    AF = mybir.ActivationFunctionType

    def r(ap):
        return ap.bitcast(f32r)

    nc = bacc.Bacc("TRN2", target_bir_lowering=False, debug=False,
                   num_devices=NCORES)

    # ---- I/O -----------------------------------------------------------
    qT_d = nc.dram_tensor("qT", [B, QD, TLOC], f32, kind="ExternalInput")
    keyT_d = nc.dram_tensor("keyT", [B, KD, S], f32, kind="ExternalInput")
    val_d = nc.dram_tensor("val", [B, S, VD], f32, kind="ExternalInput")
    WqT_d = nc.dram_tensor("WqT", [QD, E], f32, kind="ExternalInput")
    WkT_d = nc.dram_tensor("WkT", [KD, E], f32, kind="ExternalInput")
    WoT_d = nc.dram_tensor("WoT", [E, E], f32, kind="ExternalInput")
    bq_d = nc.dram_tensor("bq", [E], f32, kind="ExternalInput")
    bk_d = nc.dram_tensor("bk", [E], f32, kind="ExternalInput")
    bo_d = nc.dram_tensor("bo", [E], f32, kind="ExternalInput")
    outT_d = nc.dram_tensor("outT", [E, B, TLOC], f32, kind="ExternalOutput")
    meanT_d = nc.dram_tensor("meanT", [B, S, TLOC], f32, kind="ExternalOutput")

    with tile.TileContext(nc) as tc:
        with (
            tc.tile_pool(name="consts", bufs=1) as consts,
            tc.tile_pool(name="qT", bufs=1) as qT_pool,
            tc.tile_pool(name="ostage", bufs=2) as ostage,
            tc.tile_pool(name="dscratch", bufs=1, space="DRAM") as dpool,
        ):
            # constants
            scones = consts.tile([128, 128], f32)     # 1/H for mean fold
            nc.vector.memset(scones[:], 1.0 / H)
            ones64 = consts.tile([128, 1], f32)       # only lane 64 is used
            nc.vector.memset(ones64[:], 1.0)
            bq_sb = consts.tile([128, 8], f32)
            nc.sync.dma_start(bq_sb[:], bq_d.ap().rearrange("(c p) -> p c", p=128))
            bk_sb = consts.tile([128, 8], f32)
            nc.sync.dma_start(bk_sb[:], bk_d.ap().rearrange("(c p) -> p c", p=128))
            bo_sb = consts.tile([128, 8], f32)
            nc.sync.dma_start(bo_sb[:], bo_d.ap().rearrange("(c p) -> p c", p=128))

            kTd = dpool.tile([B, E, S], f32)     # k-proj spill (both b)
            qT_sb = qT_pool.tile([128, 16, TLOC], f32)  # idx = b*8+ec

            # ---------------- phase 1: projections ----------------------
            with (
                tc.tile_pool(name="wk8", bufs=1) as wk8,
                tc.tile_pool(name="mrhs", bufs=10) as mrhs,
                tc.tile_pool(name="qrhs", bufs=1) as qrhs,
                tc.tile_pool(name="wtq", bufs=8) as wtq,
                tc.tile_pool(name="psp", bufs=2, space="PSUM") as psp,
            ):
                # Wk^T resident: [p, kc, ec, 128]
                wk = wk8.tile([128, 8, 8, 128], f32)
                for kc in range(8):
                    nc.sync.dma_start(
                        r(wk[:, kc, :, :]),
                        r(WkT_d.ap()[kc * 128:(kc + 1) * 128, :].rearrange(
                            "p (ec f) -> p ec f", f=128)))

                # k-projection for both batches
                for b in range(B):
                    for g in range(4):          # s-groups of 512
                        rhs = []
                        for kc in range(8):
                            t_ = mrhs.tile([128, 512], f32)
                            nc.sync.dma_start(
                                r(t_[:]),
                                r(keyT_d.ap()[b, kc * 128:(kc + 1) * 128,
                                              g * 512:(g + 1) * 512]))
                            rhs.append(t_)
                        for ec in range(8):
                            ps = psp.tile([128, 512], f32)
                            for kc in range(8):
                                nc.tensor.matmul(ps[:], r(wk[:, kc, ec, :]),
                                                 r(rhs[kc][:]),
                                                 start=(kc == 0), stop=(kc == 7))
                            os_ = ostage.tile([128, 512], f32)
                            nc.scalar.activation(
                                r(os_[:]), ps[:], AF.Identity,
                                bias=bk_sb[:, ec:ec + 1])
                            nc.sync.dma_start(
                                r(kTd[b, ec * 128:(ec + 1) * 128,
                                      g * 512:(g + 1) * 512]),
                                r(os_[:]))

                # q-projection (both batches packed in free dim)
                qr = qrhs.tile([128, 16, TLOC], f32)   # idx = b*8+kc
                for b in range(B):
                    for kc in range(8):
                        nc.sync.dma_start(
                            r(qr[:, b * 8 + kc, :]),
                            r(qT_d.ap()[b, kc * 128:(kc + 1) * 128, :]))
                for ec in range(8):
                    # separate PSUM banks per batch (interleaved groups in
                    # one bank are illegal: start zeroes the whole bank)
                    psb = [psp.tile([128, 256], f32, tag="psq", name=f"psq{b_}")
                           for b_ in range(B)]
                    for kc in range(8):
                        w = wtq.tile([128, 128], f32)
                        nc.sync.dma_start(
                            r(w[:]), r(WqT_d.ap()[kc * 128:(kc + 1) * 128,
                                                  ec * 128:(ec + 1) * 128]))
                        for b in range(B):
                            nc.tensor.matmul(
                                psb[b][:], r(w[:]),
                                r(qr[:, b * 8 + kc, :]),
                                start=(kc == 0), stop=(kc == 7))
                    for b in range(B):
                        nc.scalar.activation(
                            r(qT_sb[:, b * 8 + ec, :]), psb[b][:],
                            AF.Identity, bias=bq_sb[:, ec:ec + 1])

            # ---------------- phase 2: attention -------------------------
            with (
                tc.tile_pool(name="probsT", bufs=4) as probs_pool,
                tc.tile_pool(name="khd", bufs=3) as khd_pool,
                tc.tile_pool(name="meanT", bufs=1) as mean_pool,
                tc.tile_pool(name="attnT", bufs=1) as attn_pool,
                tc.tile_pool(name="attnR", bufs=2) as attnr_pool,
                tc.tile_pool(name="ptmp", bufs=1) as ptmp_pool,
                tc.tile_pool(name="vext", bufs=1) as vext_pool,
                tc.tile_pool(name="bcs", bufs=4) as bcs_pool,
                tc.tile_pool(name="rdn", bufs=1) as rdn_pool,
                tc.tile_pool(name="wto", bufs=6) as wto,
                tc.tile_pool(name="psl", bufs=2, space="PSUM") as psl,
                tc.tile_pool(name="psav", bufs=2, space="PSUM") as psav,
                tc.tile_pool(name="pssm", bufs=2, space="PSUM") as pssm,
            ):
                def head_group(b, grp, vext, mean_b, attn_b):
                    avs = []
                    probs_l = []
                    ps_dn = pssm.tile([128, TLOC], f32, tag="sm",
                                      name="ps_dn")  # lanes 0/32/64
                    nc.vector.memset(ps_dn[:], 1.0)
                    for i, h in enumerate(grp):
                        lo = 64 * (h % 2)
                        ec = h // 2
                        # stream this head's kT slice: [d @ lo, s_tile, s_in]
                        khd = khd_pool.tile([128, 16, 128], f32, name="khd")
                        nc.sync.dma_start(
                            r(khd[lo:lo + 64, :, :]),
                            r(kTd[b, h * 64:(h + 1) * 64, :].rearrange(
                                "d (c sp) -> d c sp", sp=128)))
                        qap = qT_sb[lo:lo + 64, b * 8 + ec, :]
                        probs = probs_pool.tile([128, 16, TLOC], f32,
                                                name="probs")
                        probs_l.append(probs)
                        for quad in range(4):
                            ps = psl.tile([128, 4, TLOC], f32, name="psl_t")
                            for j in range(4):
                                st = quad * 4 + j
                                # two 1KB slices share a 2KB bank: one
                                # accumulation group per bank (start zeroes
                                # the whole bank)
                                nc.tensor.matmul(
                                    ps[:, j, :],
                                    r(khd[lo:lo + 64, st, :]),
                                    r(qap), start=(j % 2 == 0),
                                    stop=(j % 2 == 1))
                            nc.scalar.activation(
                                r(probs[:, quad * 4:(quad + 1) * 4, :]),
                                ps[:], AF.Exp, scale=SCALE)
                        av = psav.tile([128, TLOC], f32, name="av")  # [0:65]
                        for st in range(16):
                            nc.tensor.matmul(
                                av[0:65, :], r(vext[:, st, :]),
                                r(probs[:, st, :]),
                                start=(st == 0), stop=(st == 15))
                        # spill to SBUF so the PSUM slot frees early;
                        # row 64 doubles as the denominator row
                        av_sb = bcs_pool.tile([128, TLOC], f32, tag="avsb",
                                              name="av_sb")
                        avs.append(av_sb)
                        nc.vector.tensor_copy(av_sb[0:65, :], av[0:65, :])
                        # gather denominator into ps_dn lane 32*i (K=1)
                        nc.tensor.matmul(
                            ps_dn[32 * i:32 * i + 1, :], ones64[64:65, :],
                            av_sb[64:65, :],
                            start=True, stop=True)
                    rdn = rdn_pool.tile([128, TLOC], f32, name="rdn")
                    nc.vector.reciprocal(rdn[:], ps_dn[:])
                    for i, h in enumerate(grp):
                        lo = 64 * (h % 2)
                        ec = h // 2
                        av_sb = avs[i]
                        probs = probs_l[i]
                        ps_bc = pssm.tile([128, TLOC], f32, tag="sm",
                                          name="ps_bc")
                        nc.tensor.matmul(ps_bc[:],
                                         scones[32 * i:32 * i + 1, :],
                                         rdn[32 * i:32 * i + 1, :],
                                         start=True, stop=True)
                        bcs = bcs_pool.tile([128, TLOC], f32, name="bcs")
                        nc.vector.tensor_copy(bcs[:], ps_bc[:])
                        # attnT = (av * H) * bcs   (bcs carries recip/H)
                        nc.vector.scalar_tensor_tensor(
                            attn_b[lo:lo + 64, ec, :], av_sb[0:64, :],
                            float(H), bcs[0:64, :],
                            op0=mybir.AluOpType.mult,
                            op1=mybir.AluOpType.mult)
                        bb = bcs[:, None, :].broadcast_to([128, 16, TLOC])
                        if h == 0:
                            nc.vector.tensor_mul(mean_b[:], probs[:], bb)
                        else:
                            ptmp = ptmp_pool.tile([128, 16, TLOC], f32,
                                                  name="ptmp")
                            nc.vector.tensor_mul(ptmp[:], probs[:], bb)
                            nc.vector.tensor_add(mean_b[:], mean_b[:],
                                                 ptmp[:])

                def attention_batch(b):
                    vext = vext_pool.tile([128, 16, VD + 1], f32, name="vext")
                    nc.sync.dma_start(
                        r(vext[:, :, 0:VD]),
                        r(val_d.ap()[b].rearrange("(c p) v -> p c v", p=128)))
                    # ones column via ACT (f32r memset has no ISA encoding):
                    # out = Identity(in * 0 + 1) = 1
                    nc.scalar.activation(r(vext[:, :, VD:VD + 1]),
                                         scones[:, 0:16, None],
                                         AF.Identity, bias=1.0, scale=0.0)

                    mean_b = mean_pool.tile([128, 16, TLOC], f32, name="mean_b")
                    attn_b = attn_pool.tile([128, 8, TLOC], f32, name="attn_b")

                    # head groups of 3 -> denominators gathered at lanes
                    # {0, 32, 64} (the only matmul-legal base partitions),
                    # one reciprocal op per group
                    groups = [[0, 1, 2], [3, 4, 5], [6, 7, 8], [9, 10, 11],
                              [12, 13, 14], [15]]
                    for grp in groups:
                        head_group(b, grp, vext, mean_b, attn_b)
                    nc.sync.dma_start(
                        meanT_d.ap()[b].rearrange("(c p) t -> p c t", p=128),
                        mean_b[:])
                    # round attn^T to fp32r for the out-projection matmuls
                    attn_r = attnr_pool.tile([128, 8, TLOC], f32, tag="attnr",
                                             name=f"attnr{b}")
                    nc.scalar.activation(r(attn_r[:]), attn_b[:], AF.Identity)
                    return attn_r

                attnT = [attention_batch(b) for b in range(B)]

                # ---------------- phase 3: out-projection ----------------
                for ec in range(8):
                    # separate PSUM banks per batch (same bank-zero rule)
                    psb = [pssm.tile([128, 256], f32, tag="sm", name=f"pso{b_}")
                           for b_ in range(B)]
                    for cc in range(8):
                        w = wto.tile([128, 128], f32)
                        nc.sync.dma_start(
                            r(w[:]), r(WoT_d.ap()[cc * 128:(cc + 1) * 128,
                                                  ec * 128:(ec + 1) * 128]))
                        for b in range(B):
                            nc.tensor.matmul(
                                psb[b][:], r(w[:]),
                                r(attnT[b][:, cc, :]),
                                start=(cc == 0), stop=(cc == 7))
                    for b in range(B):
                        os_ = ostage.tile([128, 512], f32)
                        nc.scalar.activation(os_[:, 0:256], psb[b][:],
                                             AF.Identity,
                                             bias=bo_sb[:, ec:ec + 1])
                        nc.sync.dma_start(
                            outT_d.ap()[ec * 128:(ec + 1) * 128, b, :],
                            os_[:, 0:256])

    nc.compile()
    return nc


def get_program():
    if "nc" not in _CACHE:
        _CACHE["nc"] = _build_program()
    return _CACHE["nc"]


def make_in_maps(query, key, value, Wq, bq, Wk, bk, Wo, bo):
    """Host-side shard/layout prep. Returns per-core input dicts."""
    query = np.ascontiguousarray(np.asarray(query, dtype=np.float32))
    key = np.asarray(key, dtype=np.float32)
    value = np.asarray(value, dtype=np.float32)
    keyT = np.ascontiguousarray(key.transpose(1, 2, 0))      # (B, KD, S)
    val = np.ascontiguousarray(value.transpose(1, 0, 2))     # (B, S, VD)
    WqT = np.ascontiguousarray(np.asarray(Wq, np.float32).T)
    WkT = np.ascontiguousarray(np.asarray(Wk, np.float32).T)
    WoT = np.ascontiguousarray(np.asarray(Wo, np.float32).T)
    bq = np.ascontiguousarray(np.asarray(bq, np.float32))
    bk = np.ascontiguousarray(np.asarray(bk, np.float32))
    bo = np.ascontiguousarray(np.asarray(bo, np.float32))

    in_maps = []
    for c in range(NCORES):
        qs = query[c * TLOC:(c + 1) * TLOC]                  # (TLOC, B, QD)
        qT = np.ascontiguousarray(qs.transpose(1, 2, 0))     # (B, QD, TLOC)
        in_maps.append({
            "qT": qT, "keyT": keyT, "val": val,
            "WqT": WqT, "WkT": WkT, "WoT": WoT,
            "bq": bq, "bk": bk, "bo": bo,
        })
    return in_maps


def assemble(results):
    """Gather per-core outputs into full (attn, attn_weights_avg)."""
    attn = np.empty((T, B, E), np.float32)
    wavg = np.empty((B, T, S), np.float32)
    for c in range(NCORES):
        outT = results[c]["outT"]        # (E, B, TLOC)
        meanT = results[c]["meanT"]      # (B, S, TLOC)
        attn[c * TLOC:(c + 1) * TLOC] = outT.transpose(2, 1, 0)
        wavg[:, c * TLOC:(c + 1) * TLOC, :] = meanT.transpose(0, 2, 1)
    return attn, wavg


def kernel(query, key, value, Wq, bq, Wk, bk, Wo, bo, num_heads):
    assert int(num_heads) == H
    from concourse.bass_utils import run_bass_kernel_spmd
    nc = get_program()
    in_maps = make_in_maps(query, key, value, Wq, bq, Wk, bk, Wo, bo)
    res = run_bass_kernel_spmd(nc, in_maps, core_ids=list(range(NCORES)))
    return assemble(res.results)
